# Initial kernel scaffold
#
"""Trainium2 Bass kernel for batched multi-head attention with additive mask.

Problem (full shapes): q,k,v [2,16,2048,64] f32, mask [1,1,2048,2048] f32,
scale scalar; out = softmax(q@k^T/scale + mask) @ v -> [2,16,2048,64].

Sharding: B*H = 32 heads split over 8 cores (4 heads/core), pure data
parallel, no collectives. The shared mask is replicated to every core.

Per-core device algorithm:
  - Layout: S^T orientation. Host pre-transposes q,k to [d, seq] (q also
    pre-scaled by 1/scale, both cast to bf16) and pre-transposes the mask
    (kept f32), so every DMA is a clean contiguous pattern.
  - S^T tile [kv=128, q] = kT_tile.T @ qT  (PE bf16, contraction d=64)
  - mask add: DVE tensor_add in-place on PSUM (f32 mask -> full precision)
  - P = exp(S^T): ScalarE activation PSUM -> SBUF (bf16)
  - O'^T [65, q] = sum_kv V'[kv,65].T @ P^T[kv,q]  with V' = [ones | V]
    (bf16), accumulated in PSUM f32 over the 16 kv tiles; row 0 = softmax
    denominator.
  - PE-transpose O'^T 128-col chunks -> [q=128, 65] grouped 4 per PSUM
    bank; strided DVE reciprocal of the denominator columns + broadcast
    tensor_tensor multiply -> normalized O [q, 64] in natural layout; DMA.
"""

import numpy as np

B, H, SQ, SKV, D = 2, 16, 2048, 2048, 64
NCORES = 8
HPC = (B * H) // NCORES  # heads per core = 4
KT = SKV // 128  # kv tiles = 16
QHALF = SQ // 2  # 1024
NQJ = QHALF // 128  # 8 transpose chunks per half
DC = D + 1  # 65 cols: [denom | O]

_cached = {}


def _build():
    from concourse import bacc
    import concourse.mybir as mybir
    import concourse.tile as tile
    from concourse.masks import make_identity

    F32 = mybir.dt.float32
    BF16 = mybir.dt.bfloat16
    EXP = mybir.ActivationFunctionType.Exp
    COPY = mybir.ActivationFunctionType.Copy

    nc = bacc.Bacc("TRN2", target_bir_lowering=False, debug=False,
                   num_devices=NCORES)

    # DRAM I/O (per-core shard, host-prepped layouts)
    qT = nc.declare_dram_parameter("qT", [HPC // 2, 128, SQ], BF16, isOutput=False)
    kT = nc.declare_dram_parameter("kT", [HPC // 2, 128, SKV], BF16, isOutput=False)
    vA = nc.declare_dram_parameter("vA", [HPC, SKV, DC], BF16, isOutput=False)
    maskT = nc.declare_dram_parameter("maskT", [SKV, SQ], F32, isOutput=False)
    out = nc.declare_dram_parameter("out", [HPC, SQ, D], F32, isOutput=True)

    with tile.TileContext(nc) as tc:
        with (
            tc.tile_pool(name="mask", bufs=1) as mask_pool,
            tc.tile_pool(name="qk", bufs=1) as qk_pool,
            tc.tile_pool(name="vp", bufs=1) as v_pool,
            tc.tile_pool(name="const", bufs=1) as const_pool,
            tc.tile_pool(name="p", bufs=4) as p_pool,
            tc.tile_pool(name="sstage", bufs=4) as s_pool,
            tc.tile_pool(name="osb", bufs=2) as osb_pool,
            tc.tile_pool(name="outt", bufs=2) as out_pool,
            tc.tile_pool(name="r", bufs=4) as r_pool,
            tc.tile_pool(name="ps_s", bufs=2, space="PSUM") as ps_s,
            tc.tile_pool(name="ps_o", bufs=3, space="PSUM") as ps_o,
            tc.tile_pool(name="ps_t", bufs=1, space="PSUM") as ps_t,
        ):
            ident = const_pool.tile([128, 128], F32)
            make_identity(nc, ident[:])

            # resident mask tiles, one DMA per kv tile
            mask_sb = []
            for t in range(KT):
                mt = mask_pool.tile([128, SQ], F32, tag=f"m{t}", name=f"m{t}")
                nc.sync.dma_start(mt[:], maskT[t * 128:(t + 1) * 128, :])
                mask_sb.append(mt)

            # resident qT/kT, pair-stacked [128, seq]
            qT_sb, kT_sb = [], []
            for pr in range(HPC // 2):
                qt = qk_pool.tile([128, SQ], BF16, tag=f"q{pr}", name=f"q{pr}")
                nc.sync.dma_start(qt[:], qT[pr])
                qT_sb.append(qt)
                kt = qk_pool.tile([128, SKV], BF16, tag=f"k{pr}", name=f"k{pr}")
                nc.sync.dma_start(kt[:], kT[pr])
                kT_sb.append(kt)

            # resident V' per head: [128, kv_tile * 65]
            v_sb = []
            for h in range(HPC):
                vt = v_pool.tile([128, KT * DC], BF16, tag=f"v{h}", name=f"v{h}")
                nc.sync.dma_start(
                    vt[:].rearrange("p (t d) -> p t d", t=KT),
                    vA[h].rearrange("(t p) d -> p t d", p=128),
                )
                v_sb.append(vt)

            # PE warm-up: ~12 dense back-to-back matmuls (~7 us) to push the
            # HAM clock gate to 8/8 before the real stream starts.
            wu_ps = ps_t.tile([128, 512], F32, tag="t", name="wu")
            for w in range(12):
                nc.tensor.matmul(
                    wu_ps[:], kT_sb[0][:, :128], qT_sb[0][:, :512],
                    start=True, stop=True,
                )

            for h in range(HPC):
                pr, sub = divmod(h, 2)
                r0, r1 = sub * 64, (sub + 1) * 64
                for half in range(2):
                    q0 = half * QHALF
                    o_acc = [
                        ps_o.tile([DC, 512], F32, tag="o", name=f"o{i}")
                        for i in range(2)
                    ]
                    for t in range(KT):
                        s_ps = ps_s.tile([128, QHALF], F32, tag="s")
                        for c2 in range(2):
                            nc.tensor.matmul(
                                s_ps[:, c2 * 512:(c2 + 1) * 512],
                                kT_sb[pr][r0:r1, t * 128:(t + 1) * 128],
                                qT_sb[pr][r0:r1, q0 + c2 * 512:q0 + (c2 + 1) * 512],
                                start=True,
                                stop=True,
                            )
                        s_sb = s_pool.tile([128, QHALF], F32, tag="ss")
                        nc.vector.tensor_add(
                            out=s_sb[:], in0=s_ps[:],
                            in1=mask_sb[t][:, q0:q0 + QHALF],
                        )
                        p_t = p_pool.tile([128, QHALF], BF16, tag="p")
                        nc.scalar.activation(p_t[:], s_sb[:], EXP)
                        for c2 in range(2):
                            nc.tensor.matmul(
                                o_acc[c2][:],
                                v_sb[h][:, t * DC:(t + 1) * DC],
                                p_t[:, c2 * 512:(c2 + 1) * 512],
                                start=(t == 0),
                                stop=(t == KT - 1),
                            )
                    # normalize + transpose to natural [q, d] layout
                    o_sb = osb_pool.tile([DC, QHALF], F32, tag="osb")
                    for c2 in range(2):
                        nc.scalar.activation(
                            o_sb[:, c2 * 512:(c2 + 1) * 512], o_acc[c2][:], COPY
                        )
                    out_t = out_pool.tile([128, NQJ * D], F32, tag="out")
                    for g in range(NQJ // 4):  # groups of 4 transposes
                        t_ps = ps_t.tile([128, 4 * DC], F32, tag="t")
                        for jj in range(4):
                            j = g * 4 + jj
                            nc.tensor.transpose(
                                t_ps[:, jj * DC:(jj + 1) * DC],
                                o_sb[:, j * 128:(j + 1) * 128],
                                ident[:DC, :DC],
                            )
                        r_sb = r_pool.tile([128, 4], F32, tag="r")
                        nc.vector.reciprocal(
                            r_sb[:],
                            t_ps[:].rearrange("p (j d) -> p j d", j=4)[:, :, 0:1],
                        )
                        nc.vector.tensor_mul(
                            out=out_t[:, g * 4 * D:(g + 1) * 4 * D].rearrange(
                                "p (j d) -> p j d", j=4
                            ),
                            in0=t_ps[:].rearrange("p (j d) -> p j d", j=4)[
                                :, :, 1:DC
                            ],
                            in1=r_sb[:].rearrange("p (j o) -> p j o", o=1)
                            .broadcast_to([128, 4, D]),
                        )
                    nc.sync.dma_start(
                        out[h, q0:q0 + QHALF, :].rearrange(
                            "(j p) d -> p j d", p=128
                        ),
                        out_t[:].rearrange("p (j d) -> p j d", j=NQJ),
                    )
    nc.compile()
    return nc


def _prep_in_maps(q, k, v, mask, s):
    import ml_dtypes

    bf16 = ml_dtypes.bfloat16
    # host prep: fold 1/scale into q; transpose to [d, seq]; pair-stack heads
    qh = (q / s).reshape(B * H, SQ, D).transpose(0, 2, 1)  # [32, 64, 2048]
    kh = k.reshape(B * H, SKV, D).transpose(0, 2, 1)
    vh = v.reshape(B * H, SKV, D)
    vA = np.concatenate(
        [np.ones((B * H, SKV, 1), dtype=np.float32), vh], axis=2
    ).astype(bf16)  # [32, 2048, 65], col 0 = ones
    maskT = np.ascontiguousarray(mask.reshape(SQ, SKV).T)

    in_maps = []
    for c in range(NCORES):
        h0 = c * HPC
        qTc = np.ascontiguousarray(
            qh[h0:h0 + HPC].reshape(HPC // 2, 128, SQ)
        ).astype(bf16)
        kTc = np.ascontiguousarray(
            kh[h0:h0 + HPC].reshape(HPC // 2, 128, SKV)
        ).astype(bf16)
        vAc = np.ascontiguousarray(vA[h0:h0 + HPC])
        in_maps.append({"qT": qTc, "kT": kTc, "vA": vAc, "maskT": maskT})
    return in_maps


def kernel(q, k, v, mask, scale):
    from concourse.bass_utils import run_bass_kernel_spmd

    q = np.asarray(q, dtype=np.float32)
    k = np.asarray(k, dtype=np.float32)
    v = np.asarray(v, dtype=np.float32)
    mask = np.asarray(mask, dtype=np.float32)
    s = float(np.asarray(scale))

    in_maps = _prep_in_maps(q, k, v, mask, s)

    if "nc" not in _cached:
        _cached["nc"] = _build()
    res = run_bass_kernel_spmd(_cached["nc"], in_maps, list(range(NCORES)))

    outs = [res.results[c]["out"] for c in range(NCORES)]  # [4, 2048, 64] each
    full = np.concatenate(outs, axis=0).reshape(B, H, SQ, D)
    return full



# revision 15
# speedup vs baseline: 2.2545x; 2.2545x over previous
"""Trainium2 Bass kernel for batched multi-head attention with additive mask.

Problem (full shapes): q,k,v [2,16,2048,64] f32, mask [1,1,2048,2048] f32,
scale scalar; out = softmax(q@k^T/scale + mask) @ v -> [2,16,2048,64].

Sharding: B*H = 32 heads split over 8 cores (4 heads/core), pure data
parallel, no collectives. The shared mask is replicated to every core.

Per-core device algorithm (ScalarE-exp-bound design):
  - S^T orientation: S^T tile [kv=128, q=512] = kT.T @ qT (PE, contraction
    d=64). Host pre-transposes q (pre-scaled by 1/scale) and k to [d, seq]
    bf16. Two kv tiles are computed CONCURRENTLY by row-packing the PE
    array: even tiles' weights at array rows 0-63, odd tiles' at 64-127
    (qT duplicated to partitions 64-127) -> ~2x QK throughput.
  - The additive mask is applied multiplicatively after exp:
    P = exp(S^T) * exp(maskT), with expM = exp(mask^T) precomputed on the
    host in bf16; the bf16 multiply runs on VectorE at 2x rate.
  - exp runs on ScalarE over groups of THREE kv tiles (FD=1536): staging
    is a pool of two [128,1536] PSUM tiles (3 banks each) rotating per
    group. The 256-tile stream is grouped globally (groups may cross
    chunk boundaries; the expM multiply splits in two there). Larger FD
    amortizes the ~180-cycle per-instruction PSUM-access overhead of the
    saturated ScalarE stream; the 3-matmul refill of one buffer fits
    inside the other buffer's 1430ns ACT even with a throttled PE.
    NOTE: staging MUST be separate pool tiles - a single manually-sliced
    PSUM tile serializes the pipeline (coarse dependency tracking).
  - O'^T [65, 512] = sum_kv V'[kv,65].T @ P^T[kv,q] with V' = [ones | V]
    (bf16), accumulated in PSUM f32 over the 16 kv tiles of a (head,
    q-chunk); row 0 = softmax denominator. Drained via VectorE copy to
    SBUF and DMA'd out as [65, 2048] per head; the host divides by the
    denominator row and transposes to [2048, 64] during unsharding.
  - PSUM budget: 2 x 3-bank staging + 2 x [65,512] O accumulators = 8.
  - PE warm-up on a memset tile (no DMA dependency) pushes the HAM clock
    gate to 8/8 before the real stream; without it the whole kernel runs
    at K=4/8 and the cold PE becomes the critical path.
  - PE instruction stream is software-pipelined: QK matmuls run two ACT
    groups ahead of PV matmuls so the in-order PE queue never stalls
    behind the exp/mul chain.
"""

import numpy as np

B, H, SQ, SKV, D = 2, 16, 2048, 2048, 64
NCORES = 8
HPC = (B * H) // NCORES  # heads per core = 4
KT = SKV // 128  # kv tiles per chunk = 16
QC = 4  # q chunks per head
QCW = SQ // QC  # 512 q columns per chunk
DC = D + 1  # 65 rows: [denom | O]
NT = QC * HPC * KT  # flat kv-tile stream length = 256
GS = 3  # kv tiles per ScalarE exp group

_cached = {}


def _build():
    from concourse import bacc
    import concourse.mybir as mybir
    import concourse.tile as tile

    F32 = mybir.dt.float32
    BF16 = mybir.dt.bfloat16
    EXP = mybir.ActivationFunctionType.Exp

    nc = bacc.Bacc("TRN2", target_bir_lowering=False, debug=False,
                   num_devices=NCORES)

    # DRAM I/O (per-core shard, host-prepped layouts)
    qT2 = nc.declare_dram_parameter("qT2", [HPC, 128, SQ], BF16, isOutput=False)
    kT2 = nc.declare_dram_parameter("kT2", [HPC, 128, (KT // 2) * 128], BF16,
                                    isOutput=False)
    vA = nc.declare_dram_parameter("vA", [HPC, 128, KT * DC], BF16,
                                   isOutput=False)
    expM = nc.declare_dram_parameter("expM", [128, QC, KT, QCW], BF16,
                                     isOutput=False)
    out = nc.declare_dram_parameter("out", [HPC, DC, SQ], F32, isOutput=True)

    # flat kv-tile stream, chunk-major (c outer so expM streams by chunk);
    # tile tau -> (c, h, t_local)
    def tile_chk(tau):
        k, tl = divmod(tau, KT)
        c, h = divmod(k, HPC)
        return c, h, tl

    # groups sized [2] + 84x[3] + [2] (= 256 tiles): small first group
    # shortens the pipeline-fill chain before the first exp, small last
    # group shortens the drain chain after the final exp.
    ngroups = 86

    def group_tiles(g):
        if g == 0:
            return range(0, 2)
        if g == ngroups - 1:
            return range(254, 256)
        return range(2 + 3 * (g - 1), 2 + 3 * g)

    def tile_group_slot(t):
        if t < 2:
            return 0, t
        if t >= 254:
            return ngroups - 1, t - 254
        return (t - 2) // 3 + 1, (t - 2) % 3

    with tile.TileContext(nc) as tc:
        with (
            tc.tile_pool(name="qk", bufs=1) as qk_pool,
            tc.tile_pool(name="vp", bufs=1) as v_pool,
            tc.tile_pool(name="em", bufs=1) as em_pool,
            tc.tile_pool(name="e", bufs=3) as e_pool,
            tc.tile_pool(name="p", bufs=6) as p_pool,
            tc.tile_pool(name="osb", bufs=3) as osb_pool,
            tc.tile_pool(name="ps_h", bufs=2, space="PSUM") as ps_h,
            tc.tile_pool(name="ps_o", bufs=2, space="PSUM") as ps_o,
        ):
            qT_sb, kT_sb, v_sb = [], [], []
            for h in range(HPC):
                qT_sb.append(qk_pool.tile([128, SQ], BF16, tag=f"q{h}",
                                          name=f"q{h}"))
                kT_sb.append(qk_pool.tile([128, (KT // 2) * 128], BF16,
                                          tag=f"k{h}", name=f"k{h}"))
                v_sb.append(v_pool.tile([128, KT * DC], BF16, tag=f"v{h}",
                                        name=f"v{h}"))
            em = em_pool.tile([128, QC * KT * QCW], BF16, tag="em", name="em")
            emv = em[:].rearrange("p (c t q) -> p c t q", c=QC, t=KT)

            def load_head(h):
                nc.sync.dma_start(qT_sb[h][:], qT2[h])
                nc.sync.dma_start(kT_sb[h][:], kT2[h])
                nc.sync.dma_start(v_sb[h][:], vA[h])

            def load_expm_chunk(c):
                for b in range(KT // 2):
                    nc.sync.dma_start(
                        emv[:, c, 2 * b:2 * b + 2, :],
                        expM[:, c, 2 * b:2 * b + 2, :],
                    )

            load_head(0)
            nc.sync.dma_start(emv[:, 0, 0:2, :], expM[:, 0, 0:2, :])
            load_head(1)
            for b in range(1, KT // 2):
                nc.sync.dma_start(
                    emv[:, 0, 2 * b:2 * b + 2, :],
                    expM[:, 0, 2 * b:2 * b + 2, :],
                )
            load_head(2)
            load_head(3)
            for c in range(1, QC):
                load_expm_chunk(c)

            # PE warm-up on a memset tile (no DMA dependency): ~3.8us of
            # back-to-back matmuls flips the HAM clock gate to 8/8 while
            # the input DMAs stream in.
            wu = qk_pool.tile([64, 512], BF16, tag="wu", name="wu")
            nc.vector.memzero(wu[:])
            wu_ps = ps_h.tile([128, GS * QCW], F32, tag="h", name="wu_ps")
            for _ in range(9):
                nc.tensor.matmul(
                    wu_ps[:, 0:512], wu[:, 0:128], wu[:],
                    start=True, stop=True,
                )

            # in-flight state: ("g", g) staging tile, ("p", g) p tile,
            # (c, h) o_acc
            st = {}
            emitted_pairs = set()

            def emit_qk_group(g):
                # emit the row-packed QK pairs whose tiles land in group g
                # (a pair's odd matmul may spill into group g+1's tile)
                for tau in group_tiles(g):
                    u = tau // 2
                    if u in emitted_pairs:
                        continue
                    emitted_pairs.add(u)
                    c, h, tl = tile_chk(2 * u)
                    if tl == 0:
                        st[(c, h)] = ps_o.tile([DC, QCW], F32, tag="o",
                                               name=f"o{c}_{h}")
                    q0 = c * QCW
                    for sub in range(2):
                        t2 = 2 * u + sub
                        if t2 >= NT:
                            break
                        gg, slot = tile_group_slot(t2)
                        if ("g", gg) not in st:
                            st[("g", gg)] = ps_h.tile(
                                [128, GS * QCW], F32, tag="h", name=f"s{gg}")
                        r0, r1 = sub * 64, (sub + 1) * 64
                        nc.tensor.matmul(
                            st[("g", gg)][:, slot * QCW:(slot + 1) * QCW],
                            kT_sb[h][r0:r1,
                                     (tl // 2) * 128:(tl // 2 + 1) * 128],
                            qT_sb[h][r0:r1, q0:q0 + QCW],
                            start=True, stop=True,
                        )

            def emit_act_group(g):
                tiles = list(group_tiles(g))
                n = len(tiles) * QCW
                hb = st.pop(("g", g))
                e_t = e_pool.tile([128, GS * QCW], BF16, tag="e",
                                  name=f"e{g}")
                nc.scalar.activation(e_t[:, :n], hb[:, :n], EXP)
                p_t = p_pool.tile([128, GS * QCW], BF16, tag="p",
                                  name=f"p{g}")
                # expM multiply; split when the group crosses a chunk edge
                runs = []
                for tau in tiles:
                    c, h, tl = tile_chk(tau)
                    if runs and runs[-1][0] == c and \
                            runs[-1][1] + runs[-1][2] == tl:
                        runs[-1][2] += 1
                    else:
                        runs.append([c, tl, 1])
                off = 0
                for c, tl, ln in runs:
                    nc.vector.tensor_mul(
                        out=p_t[:, off * QCW:(off + ln) * QCW].rearrange(
                            "p (t q) -> p t q", t=ln),
                        in0=e_t[:, off * QCW:(off + ln) * QCW].rearrange(
                            "p (t q) -> p t q", t=ln),
                        in1=emv[:, c, tl:tl + ln, :],
                    )
                    off += ln
                st[("p", g)] = p_t

            def emit_pv_group(g):
                p_t = st.pop(("p", g))
                for j, tau in enumerate(group_tiles(g)):
                    c, h, tl = tile_chk(tau)
                    o_acc = st[(c, h)]
                    nc.tensor.matmul(
                        o_acc[:],
                        v_sb[h][:, tl * DC:(tl + 1) * DC],
                        p_t[:, j * QCW:(j + 1) * QCW],
                        start=(tl == 0), stop=(tl == KT - 1),
                    )
                    if tl == KT - 1:
                        o_sb = osb_pool.tile([DC, QCW], F32, tag="osb",
                                             name=f"ob{c}_{h}")
                        nc.vector.tensor_copy(o_sb[:], o_acc[:])
                        nc.sync.dma_start(
                            out[h, :, c * QCW:(c + 1) * QCW], o_sb[:]
                        )
                        del st[(c, h)]

            # software-pipelined emission: QK runs 3 ACT groups ahead of PV
            # so buffer-refill matmuls never queue behind a PV whose mul
            # input isn't ready yet
            SKEW = 3
            for g in range(ngroups + SKEW):
                if g < ngroups:
                    emit_qk_group(g)
                    emit_act_group(g)
                if g >= SKEW:
                    emit_pv_group(g - SKEW)
    nc.compile()
    return nc


def _prep_in_maps(q, k, v, mask, s):
    import ml_dtypes

    bf16 = ml_dtypes.bfloat16
    # host prep: fold 1/scale into q; transpose to [d, seq]
    qh = (q / s).reshape(B * H, SQ, D).transpose(0, 2, 1)  # [32, 64, 2048]
    kh = k.reshape(B * H, SKV, D).transpose(0, 2, 1)
    vh = v.reshape(B * H, SKV, D)
    # qT2: qT duplicated on partitions 64..127 (row-packed QK streams)
    qT2 = np.concatenate([qh, qh], axis=1).astype(bf16)  # [32, 128, 2048]
    # kT2: even kv tiles' kT on partitions 0..63, odd tiles' on 64..127
    kt = kh.reshape(B * H, D, KT // 2, 2, 128)  # [32,64,8,2,128]
    kT2 = np.ascontiguousarray(
        np.moveaxis(kt, 3, 1)  # [32, 2, 64, 8, 128]
    ).reshape(B * H, 128, (KT // 2) * 128).astype(bf16)
    # vA: [ones | V] in lhsT layout [128(kv within tile), t, 65]
    vv = np.concatenate(
        [np.ones((B * H, SKV, 1), dtype=np.float32), vh], axis=2
    ).reshape(B * H, KT, 128, DC)
    vA = np.ascontiguousarray(vv.transpose(0, 2, 1, 3)).reshape(
        B * H, 128, KT * DC).astype(bf16)
    # expM = exp(mask^T), layout [128(kv within tile), c, t, qoff]
    expm = np.exp(mask.reshape(SQ, SKV).T)  # [kv, q]
    expm = expm.reshape(KT, 128, QC, QCW).transpose(1, 2, 0, 3)
    expm = np.ascontiguousarray(expm).astype(bf16)  # [128, QC, KT, QCW]

    in_maps = []
    for cidx in range(NCORES):
        h0 = cidx * HPC
        in_maps.append({
            "qT2": np.ascontiguousarray(qT2[h0:h0 + HPC]),
            "kT2": np.ascontiguousarray(kT2[h0:h0 + HPC]),
            "vA": np.ascontiguousarray(vA[h0:h0 + HPC]),
            "expM": expm,
        })
    return in_maps


def kernel(q, k, v, mask, scale):
    from concourse.bass_utils import run_bass_kernel_spmd

    q = np.asarray(q, dtype=np.float32)
    k = np.asarray(k, dtype=np.float32)
    v = np.asarray(v, dtype=np.float32)
    mask = np.asarray(mask, dtype=np.float32)
    s = float(np.asarray(scale))

    in_maps = _prep_in_maps(q, k, v, mask, s)

    if "nc" not in _cached:
        _cached["nc"] = _build()
    res = run_bass_kernel_spmd(_cached["nc"], in_maps, list(range(NCORES)))

    outs = []
    for c in range(NCORES):
        o = res.results[c]["out"]  # [4, 65, 2048]: row 0 = denominator
        outs.append(o[:, 1:, :] / o[:, 0:1, :])
    full = np.concatenate(outs, axis=0)  # [32, 64, 2048]
    return np.ascontiguousarray(full.transpose(0, 2, 1)).reshape(
        B, H, SQ, D).astype(np.float32)
